# revision 12
# baseline (speedup 1.0000x reference)
"""Self-contained Trainium2 Bass kernel for nn_ConvLayer (GNN message passing).

kernel(**inputs) takes full unsharded numpy inputs and returns the full
[8192, 64] float32 output, running on 8 NeuronCores via bass SPMD.

v2: bf16/fp16 matmuls (4x PE speedup vs fp32), and the 35M-element
sum_i relu(q) reduction split across two engine paths:
  A path: DVE fused abs-reduce (relu(x) = (x+|x|)/2 trick)
  B path: ScalarE relu + TensorE partition-sum matmul (transposed layout)
invd2 (0.5/deg) is folded into the one-hot segment-sum weights.
"""

import sys

for _p in ("/opt/trn_rl_repo", "/root/.axon_site/_ro/trn_rl_repo"):
    if _p not in sys.path:
        sys.path.insert(0, _p)

import numpy as np
import ml_dtypes

import concourse.bass as bass
import concourse.mybir as mybir
import concourse.tile as tile
from concourse import bacc

F32 = mybir.dt.float32
BF16 = mybir.dt.bfloat16
FP16 = mybir.dt.float16
AX = mybir.AxisListType
ALU = mybir.AluOpType
AF = mybir.ActivationFunctionType
NPBF16 = ml_dtypes.bfloat16

F_IN, F_OUT, E_DIM = 256, 64, 32
KAUG = E_DIM + 1          # edge features + ones row (bias)
CW = F_OUT * F_OUT        # 4096 wide edge-MLP output
WIN = 512                 # node window width for segment matmuls
GE = 512                  # edges per group (4 blocks)
NA_FRAC = 8, 17           # A-path groups ratio (nA = round(ng * 8/17))


def _bf(x):
    return np.ascontiguousarray(x.astype(NPBF16))


def _f16(x):
    return np.ascontiguousarray(x.astype(np.float16))


def preprocess(inputs, n_cores=8):
    """Host-side sharding/packing. Returns (meta, per_core_inmaps)."""
    h_neigh = np.asarray(inputs["h_neigh"], np.float32)
    h_self = np.asarray(inputs["h_self"], np.float32)
    edge_features = np.asarray(inputs["edge_features"], np.float32)
    W_preagg = np.asarray(inputs["W_preagg"], np.float32)
    W_self = np.asarray(inputs["W_self"], np.float32)
    W_neigh = np.asarray(inputs["W_neigh"], np.float32)
    W_edge = np.asarray(inputs["W_edge"], np.float32)
    b_edge = np.asarray(inputs["b_edge"], np.float32)
    src = np.asarray(inputs["src"], np.int32)
    dst = np.asarray(inputs["dst"], np.int32)

    N = h_neigh.shape[0]
    E = src.shape[0]
    n_loc = N // n_cores
    win = min(WIN, n_loc)
    n_win = (n_loc + win - 1) // win

    order = np.argsort(dst, kind="stable")
    dst_s = dst[order]
    bounds = np.searchsorted(dst_s, np.arange(n_cores + 1) * n_loc)
    counts = np.diff(bounds)
    e_pad = int(max(GE, -(-int(counts.max()) // GE) * GE))
    nb = e_pad // 128
    ng = e_pad // GE

    deg = np.bincount(dst, minlength=N).astype(np.float32)
    invd2 = (0.5 / np.maximum(deg, 1.0)).astype(np.float32)

    # weights, shared across cores.  wa columns permuted j-major:
    # col c' = j*64 + i  <- original row-major index i*64 + j
    col = np.arange(CW)
    src_col = (col % F_OUT) * F_OUT + col // F_OUT
    w_aug_full = np.concatenate([W_edge.T, b_edge[None, :]], axis=0)  # [33, 4096]
    w_aug = _bf(w_aug_full[:, src_col])
    w_sum = _bf(w_aug_full[:, src_col].reshape(KAUG, F_OUT, F_OUT).sum(axis=2))
    # W_preagg.T [256, 64] -> [128, 2*64] (two k-chunks side by side)
    wpt = W_preagg.T
    w_preagg_p = _bf(np.concatenate([wpt[:128], wpt[128:]], axis=1))  # [128,128]
    w_self_t = _bf(W_self.T)
    w_neigh_t = _bf(W_neigh.T)
    iota = _f16(np.tile(np.arange(n_win * win, dtype=np.float32), (128, 1)))
    # MM2 indicators: for col-chunk cc (cols = j in {2cc, 2cc+1} x 64 i),
    # ind64[p, cc*64 + j] = 2.0 iff j == 2*cc + p//64.  The factor 2 converts
    # the B path's exact sum_i relu to the A path's (sum q + sum|q|) = 2s
    # convention (invd2 = 0.5/deg then applies uniformly in the one-hots).
    ind64 = np.zeros((128, 32 * F_OUT), np.float16)
    for cc in range(32):
        ind64[:64, cc * F_OUT + 2 * cc] = 2.0
        ind64[64:, cc * F_OUT + 2 * cc + 1] = 2.0
    ident = np.eye(F_OUT, dtype=np.float16)

    in_maps = []
    win_blocks_all = None
    for c in range(n_cores):
        idx = order[bounds[c]:bounds[c + 1]]
        n_c = len(idx)
        dloc = dst[idx] - c * n_loc

        edge_c = np.zeros((KAUG, e_pad), np.float32)
        edge_c[:E_DIM, :n_c] = edge_features[idx].T
        edge_c[E_DIM, :n_c] = 1.0

        hng_full = np.zeros((F_IN, e_pad), np.float32)
        hng_full[:, :n_c] = h_neigh[src[idx]].T
        # [128, 2, e_pad]: partition p, (k, e)
        hng_p = hng_full.reshape(2, 128, e_pad).transpose(1, 0, 2)
        hng_p = _bf(hng_p.reshape(128, 2 * e_pad))

        hs_full = h_self[c * n_loc:(c + 1) * n_loc].T  # [256, n_loc]
        hs_p = hs_full.reshape(2, 128, n_loc).transpose(1, 0, 2)
        hs_p = _bf(hs_p.reshape(128, 2 * n_loc))

        arr = np.full(e_pad, -1.0, np.float32)
        arr[:n_c] = dloc.astype(np.float32)
        dstloc_c = np.ascontiguousarray(arr.reshape(nb, 128).T)  # [128, nb] f32

        arr = np.zeros(e_pad, np.float32)
        arr[:n_c] = invd2[dst[idx]]
        invd_c = np.ascontiguousarray(arr.reshape(nb, 128).T)  # [128, nb] f32

        # window membership per block (on real edges only)
        wb = [[] for _ in range(n_win)]
        for b in range(nb):
            lo = b * 128
            hi = min(lo + 128, n_c)
            if hi <= lo:
                continue
            w0 = int(dloc[lo]) // win
            w1 = int(dloc[hi - 1]) // win
            for w in range(w0, w1 + 1):
                wb[w].append(b)
        for w in range(n_win):
            if not wb[w]:
                wb[w].append(nb - 1)
        if win_blocks_all is None:
            win_blocks_all = wb
        else:
            # SPMD: all cores share one program; merge block lists so the
            # program is identical (superset schedule, zero rows are no-ops).
            win_blocks_all = [sorted(set(a) | set(b2))
                              for a, b2 in zip(win_blocks_all, wb)]

        in_maps.append({
            "edge_t": _bf(edge_c),
            "hng_t": hng_p,
            "h_self_t": hs_p,
            "dstloc": dstloc_c,
            "invd": invd_c,
            "w_aug": w_aug,
            "w_sum": w_sum,
            "w_preagg_p": w_preagg_p,
            "w_self_t": w_self_t,
            "w_neigh_t": w_neigh_t,
            "iota": iota,
            "ind": ind64,
            "ident": ident,
        })

    meta = dict(n_loc=n_loc, n_win=n_win, win=win, e_pad=e_pad, nb=nb, ng=ng,
                win_blocks=win_blocks_all, n_cores=n_cores)
    return meta, in_maps


def build_program(meta, num_devices=8, repeats=1):
    n_loc, n_win, e_pad, nb, ng = (meta["n_loc"], meta["n_win"],
                                   meta["e_pad"], meta["nb"], meta["ng"])
    win_blocks = meta["win_blocks"]
    win = meta["win"]
    nt = n_loc // 128          # node tiles per core
    nA = (ng * NA_FRAC[0] + NA_FRAC[1] // 2) // NA_FRAC[1]
    # spread A groups evenly among the ng groups
    is_a = [((g + 1) * nA) // ng > (g * nA) // ng for g in range(ng)]

    nc = bacc.Bacc("TRN2", target_bir_lowering=False, debug=False,
                   enable_asserts=False, num_devices=num_devices)

    def din(name, shape, dt):
        return nc.dram_tensor(name, list(shape), dt, kind="ExternalInput").ap()

    edge_t = din("edge_t", (KAUG, e_pad), BF16)
    hng_t = din("hng_t", (128, 2 * e_pad), BF16)
    h_self_t = din("h_self_t", (128, 2 * n_loc), BF16)
    dstloc = din("dstloc", (128, nb), F32)
    invd = din("invd", (128, nb), F32)
    w_aug = din("w_aug", (KAUG, CW), BF16)
    w_sum = din("w_sum", (KAUG, F_OUT), BF16)
    w_preagg_p = din("w_preagg_p", (128, 128), BF16)
    w_self_t = din("w_self_t", (F_OUT, F_OUT), BF16)
    w_neigh_t = din("w_neigh_t", (F_OUT, F_OUT), BF16)
    iota = din("iota", (128, n_win * win), FP16)
    ind_d = din("ind", (128, 32 * F_OUT), FP16)
    ident_d = din("ident", (F_OUT, F_OUT), FP16)
    z_out = nc.dram_tensor("z_out", [n_loc, F_OUT], F32,
                           kind="ExternalOutput").ap()

    with tile.TileContext(nc) as tc:
        with (
            tc.tile_pool(name="const", bufs=1) as cpool,
            tc.tile_pool(name="big", bufs=1) as bigpool,
            tc.tile_pool(name="ps", bufs=1, space="PSUM") as pp,
            tc.tile_pool(name="sb", bufs=4) as spool,
            tc.tile_pool(name="rq", bufs=4) as rqpool,
            tc.tile_pool(name="mt", bufs=3) as mpool,
        ):
            # ---- constants (outside repeat loop) ----
            wa = cpool.tile([KAUG, CW], BF16, tag="wa")
            nc.sync.dma_start(out=wa[:], in_=w_aug[:])
            ws = cpool.tile([KAUG, F_OUT], BF16, tag="ws")
            nc.sync.dma_start(out=ws[:], in_=w_sum[:])
            wp = cpool.tile([128, 128], BF16, tag="wp")
            nc.sync.dma_start(out=wp[:], in_=w_preagg_p[:])
            wself = cpool.tile([F_OUT, F_OUT], BF16, tag="wself")
            nc.sync.dma_start(out=wself[:], in_=w_self_t[:])
            wneigh = cpool.tile([F_OUT, F_OUT], BF16, tag="wneigh")
            nc.sync.dma_start(out=wneigh[:], in_=w_neigh_t[:])
            dl = cpool.tile([128, nb], F32, tag="dl")
            nc.sync.dma_start(out=dl[:], in_=dstloc[:])
            iv = cpool.tile([128, nb], F32, tag="iv")
            nc.sync.dma_start(out=iv[:], in_=invd[:])
            io = cpool.tile([128, n_win * win], FP16, tag="io")
            nc.sync.dma_start(out=io[:], in_=iota[:])
            ind = cpool.tile([128, 32 * F_OUT], FP16, tag="ind")
            nc.sync.dma_start(out=ind[:], in_=ind_d[:])
            idt = cpool.tile([F_OUT, F_OUT], FP16, tag="idt")
            nc.sync.dma_start(out=idt[:], in_=ident_d[:])

            for _rep in range(repeats):
                # ---- streamed inputs (inside repeat: honest timing) ----
                et = cpool.tile([KAUG, e_pad], BF16, tag="et")
                nc.sync.dma_start(out=et[:], in_=edge_t[:])
                hng = cpool.tile([128, 2, e_pad], BF16, tag="hng")
                nq = e_pad // 4
                for qq in range(4):
                    nc.sync.dma_start(
                        out=hng[:, :, nq * qq:nq * (qq + 1)],
                        in_=hng_t[:].rearrange("p (k e) -> p k e", k=2)[
                            :, :, nq * qq:nq * (qq + 1)])
                hst = cpool.tile([128, 2, n_loc], BF16, tag="hst")
                nc.sync.dma_start(
                    out=hst[:], in_=h_self_t[:].rearrange("p (k e) -> p k e", k=2))

                # persistent SBUF results
                g_all = bigpool.tile([128, nb * F_OUT], FP16, tag="g")
                g_all_t = bigpool.tile([F_OUT, e_pad], FP16, tag="gt")
                msg_all = bigpool.tile([128, nb * F_OUT], FP16, tag="msg")
                hsy = bigpool.tile([F_OUT, n_loc], BF16, tag="hsy")
                neigh_t = bigpool.tile([F_OUT, n_loc], BF16, tag="neigh")

                # ---- node stage: hsY^T = relu(W_preagg @ h_self^T) ----
                for j in range(n_loc // win):
                    ps = pp.tile([F_OUT, win], F32, tag="bs", bufs=2)
                    for k in range(2):
                        nc.tensor.matmul(
                            out=ps[:], lhsT=wp[:, F_OUT * k:F_OUT * (k + 1)],
                            rhs=hst[:, k, win * j:win * (j + 1)],
                            start=(k == 0), stop=(k == 1))
                    nc.scalar.activation(hsy[:, win * j:win * (j + 1)], ps[:],
                                         AF.Relu)

                # ---- segsum + finals emitted as soon as a node window's
                # last contributing edge group is done ----
                def emit_window(w):
                    blocks = win_blocks[w]
                    ap = pp.tile([F_OUT, win], F32, tag="bs", bufs=2,
                                 name=f"ap{w}")
                    for i, b in enumerate(blocks):
                        oh = spool.tile([128, win], FP16, tag="oh",
                                        name=f"oh{w}_{b}")
                        nc.vector.tensor_scalar(
                            out=oh[:], in0=io[:, win * w:win * (w + 1)],
                            scalar1=dl[:, b:b + 1], scalar2=iv[:, b:b + 1],
                            op0=ALU.is_equal, op1=ALU.mult)
                        nc.tensor.matmul(
                            out=ap[:],
                            lhsT=msg_all[:, F_OUT * b:F_OUT * (b + 1)],
                            rhs=oh[:], start=(i == 0),
                            stop=(i == len(blocks) - 1))
                    nc.scalar.copy(neigh_t[:, win * w:win * (w + 1)], ap[:])
                    for t in range(4 * w, min(4 * w + 4, nt)):
                        p1 = pp.tile([128, F_OUT], F32, tag="tr",
                                     name=f"p1_{t}")
                        nc.tensor.matmul(out=p1[:],
                                         lhsT=hsy[:, 128 * t:128 * (t + 1)],
                                         rhs=wself[:], start=True, stop=True)
                        a1 = spool.tile([128, F_OUT], FP16, tag="a1",
                                        name=f"a1_{t}")
                        nc.scalar.activation(a1[:], p1[:], AF.Relu)
                        p2 = pp.tile([128, F_OUT], F32, tag="tr",
                                     name=f"p2_{t}")
                        nc.tensor.matmul(
                            out=p2[:], lhsT=neigh_t[:, 128 * t:128 * (t + 1)],
                            rhs=wneigh[:], start=True, stop=True)
                        a2 = spool.tile([128, F_OUT], FP16, tag="a2",
                                        name=f"a2_{t}")
                        nc.scalar.activation(a2[:], p2[:], AF.Relu)
                        zt = spool.tile([128, F_OUT], FP16, tag="zt",
                                        name=f"zt{t}")
                        nc.vector.tensor_tensor(out=zt[:], in0=a1[:],
                                                in1=a2[:], op=ALU.add)
                        zr = spool.tile([128, F_OUT], F32, tag="zr",
                                        name=f"zr{t}")
                        nc.vector.tensor_scalar_max(out=zr[:], in0=zt[:],
                                                    scalar1=0.0)
                        nc.sync.dma_start(
                            out=z_out[128 * t:128 * (t + 1), :], in_=zr[:])

                # window w ready after group containing its max block
                trigger = {}
                for w in range(n_win):
                    trigger.setdefault(max(win_blocks[w]) // 4, []).append(w)

                # ---- edge groups ----
                for g in range(ng):
                    e0 = GE * g
                    if is_a[g]:
                        # ---- A path: DVE abs-reduce ----
                        for bi in range(4):
                            b = 4 * g + bi
                            c0 = 128 * b
                            # g' = relu(hn_src @ Wpre.T), natural [128e, 64]
                            gp = pp.tile([128, F_OUT], F32, tag="qs")
                            for k in range(2):
                                nc.tensor.matmul(
                                    out=gp[:],
                                    lhsT=hng[:, k, c0:c0 + 128],
                                    rhs=wp[:, F_OUT * k:F_OUT * (k + 1)],
                                    start=(k == 0), stop=(k == 1))
                            nc.scalar.activation(
                                g_all[:, F_OUT * b:F_OUT * (b + 1)], gp[:],
                                AF.Relu)
                            # qs = sum_i q (linear part)
                            qsp = pp.tile([128, F_OUT], F32, tag="qs")
                            nc.tensor.matmul(out=qsp[:],
                                             lhsT=et[:, c0:c0 + 128],
                                             rhs=ws[:], start=True, stop=True)
                            sabs = spool.tile([128, F_OUT], F32, tag="sabs")
                            for t in range(8):
                                qp = pp.tile([128, 512], F32, tag="aq", bufs=2)
                                nc.tensor.matmul(
                                    out=qp[:], lhsT=et[:, c0:c0 + 128],
                                    rhs=wa[:, 512 * t:512 * (t + 1)],
                                    start=True, stop=True)
                                nc.vector.tensor_reduce(
                                    out=sabs[:, 8 * t:8 * (t + 1)],
                                    in_=qp[:].rearrange("p (j i) -> p j i",
                                                        i=F_OUT),
                                    axis=AX.X, op=ALU.add,
                                    apply_absolute_value=True)
                            s1 = spool.tile([128, F_OUT], F32, tag="s1")
                            nc.vector.tensor_tensor(out=s1[:], in0=qsp[:],
                                                    in1=sabs[:], op=ALU.add)
                            nc.vector.tensor_tensor(
                                out=msg_all[:, F_OUT * b:F_OUT * (b + 1)],
                                in0=s1[:],
                                in1=g_all[:, F_OUT * b:F_OUT * (b + 1)],
                                op=ALU.mult)
                    else:
                        # ---- B path: ACT relu + PE partition-sum ----
                        # g'^T = relu(Wpre @ hn_src^T)  [64, 512]
                        bsg = pp.tile([F_OUT, GE], F32, tag="bs", bufs=2)
                        for k in range(2):
                            nc.tensor.matmul(
                                out=bsg[:],
                                lhsT=wp[:, F_OUT * k:F_OUT * (k + 1)],
                                rhs=hng[:, k, e0:e0 + GE],
                                start=(k == 0), stop=(k == 1))
                        nc.scalar.activation(g_all_t[:, e0:e0 + GE], bsg[:],
                                             AF.Relu)
                        # sT[j, e] = 2 * sum_i relu(q^T), accumulated over
                        # col-chunks via indicator matmuls
                        sT = pp.tile([F_OUT, GE], F32, tag="bs", bufs=2)
                        for cc in range(32):
                            bqp = pp.tile([128, GE], F32, tag="bq", bufs=2)
                            nc.tensor.matmul(
                                out=bqp[:],
                                lhsT=wa[:, 128 * cc:128 * (cc + 1)],
                                rhs=et[:, e0:e0 + GE],
                                start=True, stop=True)
                            rq = rqpool.tile([128, GE], FP16, tag="rq")
                            nc.scalar.activation(rq[:], bqp[:], AF.Relu)
                            nc.tensor.matmul(
                                out=sT[:],
                                lhsT=ind[:, F_OUT * cc:F_OUT * (cc + 1)],
                                rhs=rq[:],
                                start=(cc == 0), stop=(cc == 31))
                        m_t = mpool.tile([F_OUT, GE], FP16, tag="mt")
                        nc.vector.tensor_tensor(
                            out=m_t[:], in0=sT[:],
                            in1=g_all_t[:, e0:e0 + GE], op=ALU.mult)
                        # transpose m^T back to [128e, 64] into msg_all
                        trp = pp.tile([128, 4 * F_OUT], FP16, tag="tr")
                        for cb in range(4):
                            nc.tensor.transpose(
                                out=trp[:, F_OUT * cb:F_OUT * (cb + 1)],
                                in_=m_t[:, 128 * cb:128 * (cb + 1)],
                                identity=idt[:])
                        nc.scalar.copy(
                            msg_all[:, F_OUT * 4 * g:F_OUT * 4 * (g + 1)],
                            trp[:])
                    for w in trigger.get(g, []):
                        emit_window(w)

    nc.compile()
    return nc


_LAST_RESULTS = None


def kernel(**inputs):
    global _LAST_RESULTS
    from concourse.bass_utils import run_bass_kernel_spmd
    meta, in_maps = preprocess(inputs, n_cores=8)
    nc = build_program(meta, num_devices=8)
    res = run_bass_kernel_spmd(nc, in_maps, core_ids=list(range(8)))
    _LAST_RESULTS = res
    return np.concatenate([np.asarray(res.results[c]["z_out"], np.float32)
                           for c in range(8)], axis=0)
